# revision 6
# baseline (speedup 1.0000x reference)
"""Distributed multi-head attention kernel for Trainium2 (8 NeuronCores).

Reference computation (EMBED=1024, HEADS=16, b=2, n=2048):
    qkv = x @ w_qkv.T                       -> [b, n, h, d, 3] (qkv innermost)
    q, k, v per head; energy = q @ k^T
    att = softmax(energy, -1) / sqrt(1024)
    out = att @ v -> [b, n, 1024]
    relu(out @ w_proj.T + b_proj)

Sharding: 2-way data parallel over batch x 4-way tensor parallel over heads.
Core c handles batch c//4, heads [4*(c%4) .. 4*(c%4)+3].  After attention,
each 4-core batch group AllGathers the per-core attention output features
(bf16) and every core computes a 256-feature slice of the output projection.

All matmuls run in 16-bit (fp16 for the x/w_qkv/q/k energy path for
precision, bf16 where exp()'s dynamic range is needed), which the PE runs at
1 cycle/row -- unlike fp32/fp32r which measured 2.2-4.4 cycles/row on HW.

Layouts (all transposed so no on-device transposes are needed):
    xT        [1024, 2048]      x[batch].T           (matmul rhs / lhsT)
    qT,kT     [64, 2048]/head   via w_qT as lhsT, xT as rhs   (fp16)
    energy^T  [k, q] tiles      via kT as lhsT, qT as rhs, psum fp32
    softmax   exp on ACT -> bf16; denominators via an extra ones-column in V
              (PV matmul computes [out^T ; sum_k exp] in one accumulation)
    out^T     [64, 2048]/head   normalized via gpsimd partition_broadcast of
              the denominator row + one DVE divide (no DRAM bounce)
    proj^T    [256, 2048]       w_projT slice as lhsT, gathered out^T as rhs

The softmax-then-scale quirk (divide by sqrt(e) AFTER softmax) is folded
into w_proj on the host (w_proj / 32).
"""

import os
import sys
import types

sys.path.insert(0, "/opt/trn_rl_repo")

import numpy as np


def _install_ntff_shim():
    """The agent image's antenv lacks axon_hooks; recreate it so
    run_bass_kernel_spmd(trace=True) can capture NTFF profiles."""
    try:
        import antenv.axon_hooks  # noqa: F401
        return
    except ImportError:
        pass
    try:
        import antenv
        from trn_agent_boot.trn_boot import _ntff_profile_via_ctypes
    except ImportError:
        return
    mod = types.ModuleType("antenv.axon_hooks")
    _hook = [None]
    mod.set_axon_ntff_profile_hook = lambda h: _hook.__setitem__(0, h)
    mod.get_axon_ntff_profile_hook = lambda: _hook[0]
    sys.modules["antenv.axon_hooks"] = mod
    antenv.axon_hooks = mod
    mod.set_axon_ntff_profile_hook(
        _ntff_profile_via_ctypes("/opt/axon/libaxon_pjrt.so")
    )


_install_ntff_shim()

import concourse.bacc as bacc
import concourse.bass as bass
import concourse.tile as tile
from concourse import mybir
from concourse.bass_utils import run_bass_kernel_spmd

B, N, E, H, D = 2, 2048, 1024, 16, 64
NCORES = 8
GROUPS = [[0, 1, 2, 3], [4, 5, 6, 7]]
HPC = H // 4            # heads per core = 4
FC = HPC * D            # attention-output features per core = 256
QKV_F = 3 * FC          # qkv features per core = 768
ET = E // 128           # 8 k-tiles over the embed dim
NT = N // 512           # 4 n-tiles of 512 (phase 1)
KT = N // 128           # 16 k-tiles of 128 over sequence
QT = N // 1024          # 2 q-tiles of 1024 (phase 2)
NCH = N // 256          # 8 output chunks of 256 columns
F32 = mybir.dt.float32
F16 = mybir.dt.float16
BF16 = mybir.dt.bfloat16

LAST_EXEC_NS = None
LAST_RESULTS = None

_CACHED_NC = None


def _build():
    nc = bacc.Bacc("TRN2", target_bir_lowering=False, num_devices=NCORES)

    xt_d = nc.dram_tensor("xt", [ET, 128, N], F16, kind="ExternalInput")
    wqkv_d = nc.dram_tensor("wqkvt", [ET, 128, QKV_F], F16, kind="ExternalInput")
    wproj_d = nc.dram_tensor("wprojt", [ET, 128, FC], BF16, kind="ExternalInput")
    bias_d = nc.dram_tensor("bias", [FC], F32, kind="ExternalInput")
    out_d = nc.dram_tensor("out", [FC, N], F32, kind="ExternalOutput")

    with tile.TileContext(nc) as tc:
        with (
            tc.tile_pool(name="persist", bufs=1) as persist,
            tc.tile_pool(name="dram", bufs=1, space="DRAM") as dram,
        ):
            # ---- persistent SBUF tensors -------------------------------
            wqkv_sb = persist.tile([128, ET, QKV_F], F16)
            for kt in range(ET):
                nc.sync.dma_start(out=wqkv_sb[:, kt, :], in_=wqkv_d[kt])
            wproj_sb = persist.tile([128, ET, FC], BF16)
            bias_sb = persist.tile([128, 2], F32)

            # q/k features of head pair p (2 heads x 64) on partitions;
            # [128, pair, n], fp16
            qt_sb = persist.tile([128, 2, N], F16)
            kt_sb = persist.tile([128, 2, N], F16)
            # v in [n, d] layout + a ones column per head: slot = [64 v | 1]
            v_sb = persist.tile([128, KT, HPC, 65], BF16)
            ones_col = nc.const_aps.tensor(1.0, [128, KT, HPC, 1], F32)
            nc.vector.tensor_copy(v_sb[:, :, :, 64:65], ones_col)

            # per-256-column-chunk DRAM bounce buffers for the AllGather
            ot_ch = [dram.tile([FC, 256], BF16, name=f"ot{i}") for i in range(NCH)]
            og_ch = [
                dram.tile([4 * FC, 256], BF16, name=f"og{i}") for i in range(NCH)
            ]

            # ---- phase 1: QKV projections ------------------------------
            with (
                tc.tile_pool(name="xtp", bufs=2) as xtp,
                tc.tile_pool(name="qkps", bufs=2, space="PSUM") as qkps_pool,
                tc.tile_pool(name="vps", bufs=2, space="PSUM") as vps_pool,
            ):
                for nt in range(NT):
                    xt_t = xtp.tile([128, ET, 512], F16, tag="xt")
                    for kt in range(ET):
                        nc.sync.dma_start(
                            out=xt_t[:, kt, :],
                            in_=xt_d[kt, :, nt * 512 : (nt + 1) * 512],
                        )
                    for pair in range(2):
                        qps = qkps_pool.tile([128, 512], F32, tag="qk")
                        for kt in range(ET):
                            nc.tensor.matmul(
                                qps[:],
                                lhsT=wqkv_sb[:, kt, pair * 128 : (pair + 1) * 128],
                                rhs=xt_t[:, kt, :],
                                start=(kt == 0),
                                stop=(kt == ET - 1),
                            )
                        nc.vector.tensor_copy(
                            qt_sb[:, pair, nt * 512 : (nt + 1) * 512], qps[:]
                        )
                        kps = qkps_pool.tile([128, 512], F32, tag="qk")
                        for kt in range(ET):
                            nc.tensor.matmul(
                                kps[:],
                                lhsT=wqkv_sb[
                                    :, kt, 256 + pair * 128 : 256 + (pair + 1) * 128
                                ],
                                rhs=xt_t[:, kt, :],
                                start=(kt == 0),
                                stop=(kt == ET - 1),
                            )
                        nc.vector.tensor_copy(
                            kt_sb[:, pair, nt * 512 : (nt + 1) * 512], kps[:]
                        )
                    for m in range(4):
                        ns = nt * 4 + m
                        vps = vps_pool.tile([128, FC], F32, tag="v")
                        for kt in range(ET):
                            nc.tensor.matmul(
                                vps[:],
                                lhsT=xt_t[:, kt, m * 128 : (m + 1) * 128],
                                rhs=wqkv_sb[:, kt, 512:768],
                                start=(kt == 0),
                                stop=(kt == ET - 1),
                            )
                        nc.vector.tensor_copy(
                            v_sb[:, ns, :, 0:64],
                            vps[:].rearrange("p (h d) -> p h d", h=HPC),
                        )

            # weights for the projection tail can load behind everything else
            for kt in range(ET):
                nc.sync.dma_start(out=wproj_sb[:, kt, :], in_=wproj_d[kt])
            nc.sync.dma_start(
                out=bias_sb, in_=bias_d[:].rearrange("(g p) -> p g", p=128)
            )

            # ---- phase 2: attention + AllGather + projection -----------
            with (
                tc.tile_pool(name="eps", bufs=2, space="PSUM") as eps_pool,
                tc.tile_pool(name="pvps", bufs=3, space="PSUM") as pvps_pool,
                tc.tile_pool(name="expp", bufs=3) as expp,
                tc.tile_pool(name="normp", bufs=4) as normp,
                tc.tile_pool(name="prjps", bufs=1, space="PSUM") as prjps_pool,
                tc.tile_pool(name="prhs", bufs=2) as prhs_pool,
                tc.tile_pool(name="outp", bufs=3) as outp,
            ):
                def emit_proj(ch):
                    # projection for one 256-column chunk (after its AllGather)
                    rhs_t = prhs_pool.tile([128, ET, 256], BF16, tag="prhs")
                    for kt in range(ET):
                        nc.sync.dma_start(
                            out=rhs_t[:, kt, :],
                            in_=og_ch[ch][kt * 128 : (kt + 1) * 128, :],
                        )
                    n_sl = slice(ch * 256, (ch + 1) * 256)
                    for mg in range(2):
                        pps = prjps_pool.tile([128, 256], F32, tag="pp")
                        for kt in range(ET):
                            nc.tensor.matmul(
                                pps[:],
                                lhsT=wproj_sb[:, kt, mg * 128 : (mg + 1) * 128],
                                rhs=rhs_t[:, kt, :],
                                start=(kt == 0),
                                stop=(kt == ET - 1),
                            )
                        ob = outp.tile([128, 256], F32, tag="ob")
                        nc.vector.tensor_scalar(
                            ob[:],
                            pps[:],
                            bias_sb[:, mg : mg + 1],
                            0.0,
                            mybir.AluOpType.add,
                            mybir.AluOpType.max,
                        )
                        nc.sync.dma_start(
                            out=out_d[mg * 128 : (mg + 1) * 128, n_sl],
                            in_=ob[:],
                        )

                for qt in range(QT):
                    q0 = qt * 1024
                    for pair in range(2):
                        for s in range(2):
                            h = 2 * pair + s
                            d_sl = slice(64 * s, 64 * (s + 1))
                            # two accumulators of [65, 512] (one PSUM bank
                            # each) so the pool can rotate them finer
                            pvs = [
                                pvps_pool.tile(
                                    [65, 512], F32, tag="pv", name=f"pv{i}"
                                )
                                for i in range(2)
                            ]
                            for kt in range(KT):
                                eps = eps_pool.tile([128, 1024], F32, tag="e")
                                for hf in range(2):
                                    nc.tensor.matmul(
                                        eps[:, hf * 512 : (hf + 1) * 512],
                                        lhsT=kt_sb[
                                            d_sl, pair, kt * 128 : (kt + 1) * 128
                                        ],
                                        rhs=qt_sb[
                                            d_sl,
                                            pair,
                                            q0 + hf * 512 : q0 + (hf + 1) * 512,
                                        ],
                                        start=True,
                                        stop=True,
                                    )
                                exp_t = expp.tile([128, 1024], BF16, tag="exp")
                                nc.scalar.activation(
                                    exp_t[:], eps[:],
                                    mybir.ActivationFunctionType.Exp,
                                )
                                for hf in range(2):
                                    nc.tensor.matmul(
                                        pvs[hf][0:65, :],
                                        lhsT=v_sb[:, kt, h, :],
                                        rhs=exp_t[:, hf * 512 : (hf + 1) * 512],
                                        start=(kt == 0),
                                        stop=(kt == KT - 1),
                                    )
                            # normalize out^T[d, q] by the ones-row sums:
                            # broadcast the denominator row across the 64
                            # partitions on gpsimd, then one DVE divide.
                            for hf in range(2):
                                pv = pvs[hf]
                                den = normp.tile([1, 512], F32, tag="den")
                                nc.vector.tensor_copy(den[:], pv[64:65, :])
                                rec = normp.tile([1, 512], F32, tag="rec")
                                nc.vector.reciprocal_approx_fast(
                                    out=rec[:], in_=den[:]
                                )
                                rec_b = normp.tile([64, 512], F32, tag="rec_b")
                                nc.gpsimd.partition_broadcast(
                                    rec_b[:], rec[:], channels=64
                                )
                                o_sb = normp.tile([64, 512], BF16, tag="o")
                                nc.vector.tensor_mul(
                                    o_sb[:], pv[0:64, :], rec_b[:]
                                )
                                for c in range(2):
                                    ch = 4 * qt + 2 * hf + c
                                    nc.sync.dma_start(
                                        out=ot_ch[ch][
                                            pair * 128 + s * 64 : pair * 128
                                            + s * 64
                                            + 64,
                                            :,
                                        ],
                                        in_=o_sb[:, c * 256 : (c + 1) * 256],
                                    )
                            # interleave last-qt's projection chunks between
                            # head sections so the PE queue never idles
                            # behind an in-flight collective
                            if qt == 1:
                                emit_proj(2 * pair + s)
                    for c in range(4):
                        ch = 4 * qt + c
                        nc.gpsimd.collective_compute(
                            "AllGather",
                            mybir.AluOpType.bypass,
                            replica_groups=GROUPS,
                            ins=[ot_ch[ch].opt()],
                            outs=[og_ch[ch].opt()],
                        )
                for c in range(4, 8):
                    emit_proj(c)

    nc.compile()
    return nc


def _get_nc():
    global _CACHED_NC
    if _CACHED_NC is None:
        _CACHED_NC = _build()
    return _CACHED_NC


def _prep_inputs(x, w_qkv, w_proj, b_proj):
    """Shard + relayout the full inputs for the 8 cores."""
    np_f16 = np.dtype(mybir.dt.np(F16))
    np_bf16 = np.dtype(mybir.dt.np(BF16))
    x = np.asarray(x, dtype=np.float32)
    w_qkv = np.asarray(w_qkv, dtype=np.float32)
    w_proj = np.asarray(w_proj, dtype=np.float32)
    b_proj = np.asarray(b_proj, dtype=np.float32)

    # x^T per batch: [E, N] -> tiles [ET, 128, N]
    xts = [
        np.ascontiguousarray(x[b].T).reshape(ET, 128, N).astype(np_f16)
        for b in range(B)
    ]
    # w_qkv rows are (h, d, qkv)-interleaved with qkv innermost
    wr = w_qkv.reshape(H, D, 3, E)
    # fold the post-softmax 1/sqrt(E) scaling into w_proj
    wp = w_proj / np.sqrt(E).astype(np.float32)

    wqkv_shards, wproj_shards, bias_shards = [], [], []
    for r in range(4):
        heads = range(4 * r, 4 * r + 4)
        qrows = np.concatenate([wr[h, :, 0, :] for h in heads], 0)  # [256, E]
        krows = np.concatenate([wr[h, :, 1, :] for h in heads], 0)
        vrows = np.concatenate([wr[h, :, 2, :] for h in heads], 0)
        w_core = np.concatenate([qrows, krows, vrows], 0)  # [768, E]
        wqkv_shards.append(
            np.ascontiguousarray(w_core.T).reshape(ET, 128, QKV_F).astype(np_f16)
        )
        wproj_shards.append(
            np.ascontiguousarray(wp[r * FC : (r + 1) * FC, :].T)
            .reshape(ET, 128, FC)
            .astype(np_bf16)
        )
        bias_shards.append(np.ascontiguousarray(b_proj[r * FC : (r + 1) * FC]))

    in_maps = []
    for c in range(NCORES):
        b, r = c // 4, c % 4
        in_maps.append(
            {
                "xt": xts[b],
                "wqkvt": wqkv_shards[r],
                "wprojt": wproj_shards[r],
                "bias": bias_shards[r],
            }
        )
    return in_maps


def kernel(x, w_qkv, w_proj, b_proj):
    global LAST_EXEC_NS, LAST_RESULTS
    nc = _get_nc()
    in_maps = _prep_inputs(x, w_qkv, w_proj, b_proj)
    trace = bool(int(os.environ.get("BASS_KERNEL_TRACE", "0")))
    res = run_bass_kernel_spmd(
        nc, in_maps, list(range(NCORES)), trace=trace
    )
    LAST_EXEC_NS = res.exec_time_ns
    LAST_RESULTS = res

    out = np.empty((B, N, E), dtype=np.float32)
    for g in range(B):
        pt = np.concatenate(
            [res.results[4 * g + r]["out"] for r in range(4)], axis=0
        )  # [1024 f, 2048 n]
        out[g] = pt.T
    return out
